# revision 33
# baseline (speedup 1.0000x reference)
"""Single-head causal attention (B=4, S=4096, D=1024, H=64) on 8 TRN2 NeuronCores.

Sharding: 2 cores per batch. Query rows are split between the pair by
interleaving 128-row blocks (core j takes blocks with parity j). The host
pair-swaps the columns of x^T for odd cores so every core runs the IDENTICAL
instruction stream (SPMD); causal asymmetry is absorbed into a tiny
(128, 2, 128) per-core mask constant.

DMA discipline: the hardware DGE rings are in-order, so a ring carrying a
transfer that WAITS on upstream compute head-of-line-blocks everything behind
it. The Sync ring therefore carries only wait-free bulk (weights, x pairs
1-3) plus tail outputs; x pair 0 rides the Scalar ring. Every tensor that
needs partition-shifted copies is instead produced directly at the right
partitions by col-tiled projection matmuls.

Device algorithm per core (f32 PSUM accumulate):
  per s-pair p (emitted as ~1-2us work units interleaved into the PREVIOUS
  attention group's scalar-bound visit cycles, so TensorE projects pair p+1
  while ScalarE exponentiates group p):
    [K^T;V^T] = [Wk|Wv]^T @ x^T  (16 full-width matmuls)
    Q^T duplicated onto both partition halves by col-tiled matmul pairs
    (PE cols 0-63 / 64-127 stream concurrently)
    odd-chunk K^T copied to partitions 64-127 via the Scalar DMA ring
    vn (V natural) via PE-transpose into a shared PSUM ring slot -> SBUF.
  attention for q-tile qi (after proj pair qi), chunk-pair u, lag-1 pipeline:
    scores: TWO row-tiled concurrent matmuls (PE rows 0-63: even chunk from
            kvt, rows 64-127: odd chunk from k2hi) -> (128, 2, N) PSUM slab
    P = exp(S/8) -> f16 (ScalarE; scores bounded, no max subtraction)
    diagonal slabs: first-block mask multiply (DVE)
    den (128, 2, 512) f16 += P (DVE), reduced by one ones-matmul per group
    AV: TWO col-tiled concurrent matmuls (PE cols 0-63: even chunk -> o_acc
        rows 0-63, cols 64-127: odd -> rows 64-127)
  out[qi] = (129, 512) f32: rows 0-127 raw o_acc halves, row 128 den.
  Host: O = (rows 0:64 + rows 64:128) / row 128, transpose to (q, h).
"""

import sys

for _p in ("/opt/trn_rl_repo", "/root/.axon_site"):
    if _p not in sys.path:
        sys.path.insert(0, _p)

import numpy as np
import ml_dtypes

B, S, D, H = 4, 4096, 1024, 64
N_CORES = 8
DC = D // 128          # 8 d-chunks
ST = S // 512          # 8 s-tiles of 512
SP = ST // 2           # 4 s-pairs of 1024
NKC = S // 128         # 32 k-chunks of 128
NQT = 4                # q-tiles of 512 per core
SCALE = 1.0 / 8.0      # 1/sqrt(H)

BF16 = ml_dtypes.bfloat16

_cached = {}


def _build_nc():
    from concourse import bacc, tile, mybir
    from concourse.masks import make_identity

    f32 = mybir.dt.float32
    bf16 = mybir.dt.bfloat16
    f16 = mybir.dt.float16

    nc = bacc.Bacc("TRN2", target_bir_lowering=False, debug=False,
                   num_devices=N_CORES)

    xT = nc.declare_dram_parameter("xT", [SP, 128, DC, 1024], bf16, isOutput=False)
    wkv = nc.declare_dram_parameter("wkv", [128, DC, 128], bf16, isOutput=False)
    wq = nc.declare_dram_parameter("wq", [128, DC, H], bf16, isOutput=False)
    bkv = nc.declare_dram_parameter("bkv", [128, 1], f32, isOutput=False)
    bkk = nc.declare_dram_parameter("bkk", [128, 1], f32, isOutput=False)
    bqd = nc.declare_dram_parameter("bqd", [128, 1], f32, isOutput=False)
    msk = nc.declare_dram_parameter("msk", [128, 2, 128], f16, isOutput=False)
    out = nc.declare_dram_parameter("out", [NQT, 129, 512], f32, isOutput=True)

    with tile.TileContext(nc) as tc:
        with (
            tc.tile_pool(name="consts", bufs=1) as consts,
            tc.tile_pool(name="xtp", bufs=1) as xtp,
            tc.tile_pool(name="kvtp", bufs=1) as kvtp,
            tc.tile_pool(name="khip", bufs=1) as khip,
            tc.tile_pool(name="vnp", bufs=1) as vnp,
            tc.tile_pool(name="qtp", bufs=1) as qtp,
            tc.tile_pool(name="ptp", bufs=6) as ptp,
            tc.tile_pool(name="denp", bufs=2) as denp,
            tc.tile_pool(name="osbp", bufs=2) as osbp,
            tc.tile_pool(name="pproj", bufs=1, space="PSUM") as pproj,
            tc.tile_pool(name="pmm", bufs=2, space="PSUM") as pmm,
            tc.tile_pool(name="pacc", bufs=2, space="PSUM") as pacc,
        ):
            wkv_sb = consts.tile([128, DC, 128], bf16)
            nc.sync.dma_start(out=wkv_sb[:], in_=wkv[:, :, :])
            bkv_sb = consts.tile([128, 1], f32)
            nc.sync.dma_start(out=bkv_sb[:], in_=bkv[:, :])
            bkk_sb = consts.tile([128, 1], f32)
            nc.sync.dma_start(out=bkk_sb[:], in_=bkk[:, :])
            bqd_sb = consts.tile([128, 1], f32)
            nc.sync.dma_start(out=bqd_sb[:], in_=bqd[:, :])
            wq_sb = consts.tile([128, DC, H], bf16)
            nc.sync.dma_start(out=wq_sb[:], in_=wq[:, :, :])
            msk_sb = consts.tile([128, 2, 128], f16)
            nc.sync.dma_start(out=msk_sb[:], in_=msk[:, :, :])
            ones_sb = consts.tile([128, 1], f16)
            nc.vector.memset(ones_sb[:], 1.0)
            ident = consts.tile([128, 128], f16)
            make_identity(nc, ident[:, :])

            kvt = {}   # s-tile -> (128, 512) f16: rows 0-63 K^T, 64-127 V^T
            k2hi = {}  # s-tile -> (128, 2, 128) f16 rows 64+: odd-chunk K^T
            vn = {}    # s-tile -> (128, 4, 64) f16 V natural
            qt = {}    # qi -> (128, 512) f16 Q^T on both partition halves

            def dma_x(p, nsplit=1, eng=None):
                eng = eng or nc.sync
                xp = xtp.tile([128, DC, 1024], bf16, tag=f"x{p}", name=f"x{p}")
                w = 1024 // nsplit
                for d in range(DC):
                    for h in range(nsplit):
                        eng.dma_start(out=xp[:, d, h * w:(h + 1) * w],
                                      in_=xT[p, :, d, h * w:(h + 1) * w])
                return xp

            xtiles = {}

            def kv_units(p, sh):
                xp = xtiles[p]
                if True:
                    s = 2 * p + sh
                    ps_kv = pproj.tile([128, 512], f32, tag="kv", name="pskv")
                    for d in range(DC):
                        nc.tensor.matmul(ps_kv[:], wkv_sb[:, d, :],
                                         xp[:, d, sh * 512:(sh + 1) * 512],
                                         start=(d == 0), stop=(d == DC - 1))
                    yield
                    kvt_s = kvtp.tile([128, 512], f16, tag=f"kvt{s}",
                                      name=f"kvt{s}")
                    nc.vector.tensor_scalar_add(kvt_s[:], ps_kv[:], bkv_sb[:, :])
                    kvt[s] = kvt_s
                    # V natural via PE transpose (PSUM ring slot -> SBUF)
                    ps_t = pmm.tile([128, 4, 64], f16, tag="mm", name="pst")
                    for cc in range(4):
                        nc.tensor.transpose(ps_t[:, cc, :],
                                            kvt_s[64:128, cc * 128:(cc + 1) * 128],
                                            ident[64:128, 64:128])
                    vn_s = vnp.tile([128, 4, 64], f16, tag=f"vn{s}",
                                    name=f"vn{s}")
                    nc.vector.tensor_copy(out=vn_s[:], in_=ps_t[:])
                    vn[s] = vn_s
                    # odd-chunk K^T at partitions 64-127 (scalar ring is
                    # empty after x0, so this never head-of-line blocks)
                    kh = khip.tile([128, 2, 128], f16, tag=f"kh{s}",
                                   name=f"kh{s}")
                    ksrc = kvt_s[0:64, :].rearrange("p (c e a) -> p c e a",
                                                    c=2, e=2)[:, :, 1, :]
                    nc.scalar.dma_start(out=kh[64:128, :, :], in_=ksrc)
                    k2hi[s] = kh
                    yield

            def q_units(p):
                # Q-lo (PE cols 0-63) || Q-hi dup (PE cols 64-127)
                xp = xtiles[p]
                ps_qk = pproj.tile([128, 512], f32, tag="q", name="psqk")
                for d in range(DC):
                    rhs_e = xp[:, d, :].rearrange("p (b e c) -> p e b c",
                                                  e=2, c=128)[:, 0, :, :]
                    st, sp = (d == 0), (d == DC - 1)
                    nc.tensor.matmul(ps_qk[0:64, :], wq_sb[:, d, :],
                                     rhs_e, start=st, stop=sp)
                    nc.tensor.matmul(ps_qk[64:128, :], wq_sb[:, d, :],
                                     rhs_e, start=st, stop=sp)
                    if d == 3:
                        yield
                yield
                qt_p = qtp.tile([128, 512], f16, tag=f"qt{p}", name=f"qt{p}")
                nc.vector.tensor_scalar_add(qt_p[0:64, :], ps_qk[0:64, :],
                                            bqd_sb[0:64, :])
                nc.vector.tensor_scalar_add(qt_p[64:128, :], ps_qk[64:128, :],
                                            bqd_sb[64:128, :])
                qt[p] = qt_p
                yield

            def proj_units(p):
                """Projection work units (~1-2us tensor each) for pair p,
                interleaved into the previous attention group's scalar-bound
                visit cycles."""
                yield from kv_units(p, 0)
                yield from kv_units(p, 1)
                yield from q_units(p)

            def run_units(g):
                if g is None:
                    return
                for _ in g:
                    pass

            def attn_group(qi, bg=None):
                o_acc = pacc.tile([128, 512], f32, tag="oacc", name=f"oacc{qi}")
                den = denp.tile([128, 2, 512], f16, tag="den", name=f"den{qi}")
                nc.vector.memset(den[:], 0.0)
                nvis = 4 * qi + 4
                pend = []

                def emit_av(v):
                    u, su, n, pt = v
                    first, last = (u == 0), (u == nvis - 1)
                    for i in range(2):
                        c = 2 * u + i
                        s, cc = c // 4, c % 4
                        nc.tensor.matmul(
                            o_acc[i * 64:(i + 1) * 64, su * 128:512],
                            vn[s][:, cc, :], pt[:, i, 0:n],
                            start=first, stop=last)

                for u in range(nvis):
                    su = max(0, u - 4 * qi)
                    n = (4 - su) * 128
                    diag = u >= 4 * qi
                    ps_s = pmm.tile([128, 2, 512], f32, tag="mm", name="pss")
                    ce, co = 2 * u, 2 * u + 1
                    se, a = ce // 4, (ce % 4) * 128
                    nc.tensor.matmul(ps_s[:, 0, 0:n],
                                     kvt[se][0:64, a:a + 128],
                                     qt[qi][0:64, su * 128:512],
                                     start=True, stop=True)
                    nc.tensor.matmul(ps_s[:, 1, 0:n],
                                     k2hi[co // 4][64:128, (co % 4) // 2, :],
                                     qt[qi][64:128, su * 128:512],
                                     start=True, stop=True)
                    pt = ptp.tile([128, 2, 512], f16, tag="pt", name="pt")
                    nc.scalar.activation(pt[:, :, 0:n], ps_s[:, :, 0:n],
                                         func=mybir.ActivationFunctionType.Exp,
                                         scale=SCALE)
                    if diag:
                        nc.vector.tensor_mul(pt[:, 0, 0:128], pt[:, 0, 0:128],
                                             msk_sb[:, 0, :])
                        nc.vector.tensor_mul(pt[:, 1, 0:128], pt[:, 1, 0:128],
                                             msk_sb[:, 1, :])
                    nc.vector.tensor_add(den[:, :, su * 128:512],
                                         den[:, :, su * 128:512],
                                         pt[:, :, 0:n])
                    pend.append((u, su, n, pt))
                    if bg is not None:
                        next(bg, None)
                        if nvis <= 4:
                            next(bg, None)
                    if len(pend) > 1:
                        emit_av(pend.pop(0))
                while pend:
                    emit_av(pend.pop(0))
                if bg is not None:
                    for _ in bg:
                        pass

                ps_den = pmm.tile([1, 512], f32, tag="mm", name="psden")
                for i in range(2):
                    nc.tensor.matmul(ps_den[:], ones_sb[:], den[:, i, :],
                                     start=(i == 0), stop=(i == 1))
                o_sb = osbp.tile([128, 512], f32, tag="osb", name="osb")
                nc.vector.tensor_copy(out=o_sb[:], in_=o_acc[:])
                den_sb = osbp.tile([1, 512], f32, tag="densb", name="densb")
                nc.vector.tensor_copy(out=den_sb[:], in_=ps_den[:])
                nc.sync.dma_start(out=out[qi, 0:128, :], in_=o_sb[:])
                nc.sync.dma_start(out=out[qi, 128:129, :], in_=den_sb[:])

            from itertools import chain
            xtiles[0] = dma_x(0, nsplit=2, eng=nc.scalar)
            xtiles[1] = dma_x(1)
            # HAM warm-up primer: tiny matmuls consuming each x0 piece as it
            # lands keep the PE active through the DMA-wait window, so the
            # clock gate is at 8/8 when real projection matmuls begin.
            warm = pacc.tile([128, 512], f32, tag="oacc", name="warm")
            xp0 = xtiles[0]
            for dd in range(DC):
                for h in range(2):
                    nc.tensor.matmul(warm[0:64, 0:64],
                                     wkv_sb[:, 0, 0:64],
                                     xp0[:, dd, h * 512:h * 512 + 64],
                                     start=True, stop=True)
            run_units(proj_units(0))
            xtiles[2] = dma_x(2)
            xtiles[3] = dma_x(3)
            attn_group(0, proj_units(1))
            attn_group(1, proj_units(2))
            attn_group(2, proj_units(3))
            attn_group(3)

    nc.compile()
    return nc


def get_nc():
    if "nc" not in _cached:
        _cached["nc"] = _build_nc()
    return _cached["nc"]


def _masks_for_half(j):
    """(128, 2, 128) first-block mask patterns, indexed by chunk parity.

    slot 0 (even local chunk): tril for both cores.
    slot 1 (odd local chunk): zeros for core 0 (block strictly above the
    diagonal there), ones for core 1 (strictly below).
    """
    p = np.arange(128)[:, None]
    jj = np.arange(128)[None, :]
    tril = (jj >= p).astype(np.float32)        # K^T layout: k on partitions
    m = np.empty((128, 2, 128), dtype=np.float32)
    m[:, 0, :] = tril
    m[:, 1, :] = 1.0 if j == 1 else 0.0
    return m.astype(np.float16)


def prepare_in_maps(x, Wk, bk, Wq, bq, Wv, bv):
    wkv = np.ascontiguousarray(
        np.concatenate([Wk, Wv], axis=1).reshape(DC, 128, 128)
        .transpose(1, 0, 2)).astype(BF16)
    wq_c = np.ascontiguousarray(
        Wq.reshape(DC, 128, H).transpose(1, 0, 2)).astype(BF16)
    bkv = np.concatenate([bk, bv]).reshape(128, 1).astype(np.float32)
    bkk = np.concatenate([bk, bk]).reshape(128, 1).astype(np.float32)
    bqd = np.concatenate([bq, bq]).reshape(128, 1).astype(np.float32)
    masks = [_masks_for_half(0), _masks_for_half(1)]

    swap = np.arange(NKC).reshape(-1, 2)[:, ::-1].reshape(-1)  # pair-swap blocks
    in_maps = []
    for core in range(N_CORES):
        b, j = core // 2, core % 2
        xTb = x[b].T                                          # (D, S)
        if j == 1:
            xTb = xTb.reshape(D, NKC, 128)[:, swap, :].reshape(D, S)
        # tile layout: (s-pair, 128, d-chunk, 1024) contiguous
        xTb = np.ascontiguousarray(
            xTb.reshape(DC, 128, SP, 1024).transpose(2, 1, 0, 3)
        ).astype(BF16)
        in_maps.append({
            "xT": xTb, "wkv": wkv, "wq": wq_c, "bkv": bkv, "bkk": bkk,
            "bqd": bqd, "msk": masks[j],
        })
    return in_maps


def assemble_output(results):
    """results: list of 8 dicts with 'out' (NQT, 129, 512) -> full (B, S, H)."""
    out = np.empty((B, S, H), dtype=np.float32)
    for core in range(N_CORES):
        b, j = core // 2, core % 2
        loc = results[core]["out"]                       # (NQT, 129, 512)
        o = (loc[:, 0:64, :] + loc[:, 64:128, :]) / loc[:, 128:129, :]
        ob = o.reshape(NQT, H, 4, 128).transpose(0, 2, 3, 1)  # (qi, bi, 128, H)
        full = out[b].reshape(NKC, 128, H)
        for qi in range(NQT):
            for bi in range(4):
                full[8 * qi + 2 * bi + j] = ob[qi, bi]
    return out


def run_sharded(inputs, trace=False, trace_kwargs=None):
    from concourse.bass_utils import run_bass_kernel_spmd

    x = np.asarray(inputs["x"], dtype=np.float32)
    in_maps = prepare_in_maps(
        x,
        np.asarray(inputs["Wk"], dtype=np.float32),
        np.asarray(inputs["bk"], dtype=np.float32),
        np.asarray(inputs["Wq"], dtype=np.float32),
        np.asarray(inputs["bq"], dtype=np.float32),
        np.asarray(inputs["Wv"], dtype=np.float32),
        np.asarray(inputs["bv"], dtype=np.float32),
    )
    nc = get_nc()
    kw = {}
    if trace:
        kw["trace"] = True
        if trace_kwargs:
            kw.update(trace_kwargs)
    res = run_bass_kernel_spmd(nc, in_maps, core_ids=list(range(N_CORES)), **kw)
    return assemble_output(res.results), res


def kernel(**inputs):
    out, _ = run_sharded(inputs)
    return out


# revision 34
# speedup vs baseline: 1.0375x; 1.0375x over previous
"""Single-head causal attention (B=4, S=4096, D=1024, H=64) on 8 TRN2 NeuronCores.

Sharding: 2 cores per batch. Query rows are split between the pair by
interleaving 128-row blocks (core j takes blocks with parity j). The host
pair-swaps the columns of x^T for odd cores so every core runs the IDENTICAL
instruction stream (SPMD); causal asymmetry is absorbed into a tiny
(128, 2, 128) per-core mask constant.

DMA discipline: the hardware DGE rings are in-order, so a ring carrying a
transfer that WAITS on upstream compute head-of-line-blocks everything behind
it. The Sync ring therefore carries only wait-free bulk (weights, x pairs
1-3) plus tail outputs; x pair 0 rides the Scalar ring. Every tensor that
needs partition-shifted copies is instead produced directly at the right
partitions by col-tiled projection matmuls.

Device algorithm per core (f32 PSUM accumulate):
  per s-pair p (emitted as ~1-2us work units interleaved into the PREVIOUS
  attention group's scalar-bound visit cycles, so TensorE projects pair p+1
  while ScalarE exponentiates group p):
    [K^T;V^T] = [Wk|Wv]^T @ x^T  (16 full-width matmuls)
    Q^T duplicated onto both partition halves by col-tiled matmul pairs
    (PE cols 0-63 / 64-127 stream concurrently)
    odd-chunk K^T copied to partitions 64-127 via the Scalar DMA ring
    vn (V natural) via PE-transpose into a shared PSUM ring slot -> SBUF.
  attention for q-tile qi (after proj pair qi), chunk-pair u, lag-1 pipeline:
    scores: TWO row-tiled concurrent matmuls (PE rows 0-63: even chunk from
            kvt, rows 64-127: odd chunk from k2hi) -> (128, 2, N) PSUM slab
    P = exp(S/8) -> f16 (ScalarE; scores bounded, no max subtraction)
    diagonal slabs: first-block mask multiply (DVE)
    den (128, 2, 512) f16 += P (DVE), reduced by one ones-matmul per group
    AV: TWO col-tiled concurrent matmuls (PE cols 0-63: even chunk -> o_acc
        rows 0-63, cols 64-127: odd -> rows 64-127)
  out[qi] = (129, 512) f32: rows 0-127 raw o_acc halves, row 128 den.
  Host: O = (rows 0:64 + rows 64:128) / row 128, transpose to (q, h).
"""

import sys

for _p in ("/opt/trn_rl_repo", "/root/.axon_site"):
    if _p not in sys.path:
        sys.path.insert(0, _p)

import numpy as np
import ml_dtypes

B, S, D, H = 4, 4096, 1024, 64
N_CORES = 8
DC = D // 128          # 8 d-chunks
ST = S // 512          # 8 s-tiles of 512
SP = ST // 2           # 4 s-pairs of 1024
NKC = S // 128         # 32 k-chunks of 128
NQT = 4                # q-tiles of 512 per core
SCALE = 1.0 / 8.0      # 1/sqrt(H)

BF16 = ml_dtypes.bfloat16

_cached = {}


def _build_nc():
    from concourse import bacc, tile, mybir
    from concourse.masks import make_identity

    f32 = mybir.dt.float32
    bf16 = mybir.dt.bfloat16
    f16 = mybir.dt.float16

    nc = bacc.Bacc("TRN2", target_bir_lowering=False, debug=False,
                   num_devices=N_CORES)

    xT = nc.declare_dram_parameter("xT", [SP, 128, DC, 1024], bf16, isOutput=False)
    wkv = nc.declare_dram_parameter("wkv", [128, DC, 128], bf16, isOutput=False)
    wq = nc.declare_dram_parameter("wq", [128, DC, H], bf16, isOutput=False)
    bkv = nc.declare_dram_parameter("bkv", [128, 1], f32, isOutput=False)
    bkk = nc.declare_dram_parameter("bkk", [128, 1], f32, isOutput=False)
    bqd = nc.declare_dram_parameter("bqd", [128, 1], f32, isOutput=False)
    msk = nc.declare_dram_parameter("msk", [128, 2, 128], f16, isOutput=False)
    out = nc.declare_dram_parameter("out", [NQT, 129, 512], f32, isOutput=True)

    with tile.TileContext(nc) as tc:
        with (
            tc.tile_pool(name="consts", bufs=1) as consts,
            tc.tile_pool(name="xtp", bufs=1) as xtp,
            tc.tile_pool(name="kvtp", bufs=1) as kvtp,
            tc.tile_pool(name="khip", bufs=1) as khip,
            tc.tile_pool(name="vnp", bufs=1) as vnp,
            tc.tile_pool(name="qtp", bufs=1) as qtp,
            tc.tile_pool(name="ptp", bufs=6) as ptp,
            tc.tile_pool(name="denp", bufs=2) as denp,
            tc.tile_pool(name="osbp", bufs=2) as osbp,
            tc.tile_pool(name="pproj", bufs=1, space="PSUM") as pproj,
            tc.tile_pool(name="pmm", bufs=2, space="PSUM") as pmm,
            tc.tile_pool(name="pacc", bufs=2, space="PSUM") as pacc,
        ):
            wkv_sb = consts.tile([128, DC, 128], bf16)
            nc.sync.dma_start(out=wkv_sb[:], in_=wkv[:, :, :])
            bkv_sb = consts.tile([128, 1], f32)
            nc.sync.dma_start(out=bkv_sb[:], in_=bkv[:, :])
            bkk_sb = consts.tile([128, 1], f32)
            nc.sync.dma_start(out=bkk_sb[:], in_=bkk[:, :])
            bqd_sb = consts.tile([128, 1], f32)
            nc.sync.dma_start(out=bqd_sb[:], in_=bqd[:, :])
            wq_sb = consts.tile([128, DC, H], bf16)
            nc.sync.dma_start(out=wq_sb[:], in_=wq[:, :, :])
            msk_sb = consts.tile([128, 2, 128], f16)
            nc.sync.dma_start(out=msk_sb[:], in_=msk[:, :, :])
            ones_sb = consts.tile([128, 1], f16)
            nc.vector.memset(ones_sb[:], 1.0)
            ident = consts.tile([128, 128], f16)
            make_identity(nc, ident[:, :])

            kvt = {}   # s-tile -> (128, 512) f16: rows 0-63 K^T, 64-127 V^T
            k2hi = {}  # s-tile -> (128, 2, 128) f16 rows 64+: odd-chunk K^T
            vn = {}    # s-tile -> (128, 4, 64) f16 V natural
            qt = {}    # qi -> (128, 512) f16 Q^T on both partition halves

            def dma_x(p, nsplit=1, eng=None):
                eng = eng or nc.sync
                xp = xtp.tile([128, DC, 1024], bf16, tag=f"x{p}", name=f"x{p}")
                w = 1024 // nsplit
                for d in range(DC):
                    for h in range(nsplit):
                        eng.dma_start(out=xp[:, d, h * w:(h + 1) * w],
                                      in_=xT[p, :, d, h * w:(h + 1) * w])
                return xp

            xtiles = {}

            def kv_units(p, sh):
                xp = xtiles[p]
                if True:
                    s = 2 * p + sh
                    ps_kv = pproj.tile([128, 512], f32, tag="kv", name="pskv")
                    for d in range(DC):
                        nc.tensor.matmul(ps_kv[:], wkv_sb[:, d, :],
                                         xp[:, d, sh * 512:(sh + 1) * 512],
                                         start=(d == 0), stop=(d == DC - 1))
                    yield
                    kvt_s = kvtp.tile([128, 512], f16, tag=f"kvt{s}",
                                      name=f"kvt{s}")
                    nc.vector.tensor_scalar_add(kvt_s[:], ps_kv[:], bkv_sb[:, :])
                    kvt[s] = kvt_s
                    # V natural via PE transpose (PSUM ring slot -> SBUF)
                    ps_t = pmm.tile([128, 4, 64], f16, tag="mm", name="pst")
                    for cc in range(4):
                        nc.tensor.transpose(ps_t[:, cc, :],
                                            kvt_s[64:128, cc * 128:(cc + 1) * 128],
                                            ident[64:128, 64:128])
                    vn_s = vnp.tile([128, 4, 64], f16, tag=f"vn{s}",
                                    name=f"vn{s}")
                    nc.vector.tensor_copy(out=vn_s[:], in_=ps_t[:])
                    vn[s] = vn_s
                    # odd-chunk K^T at partitions 64-127 (scalar ring is
                    # empty after x0, so this never head-of-line blocks)
                    kh = khip.tile([128, 2, 128], f16, tag=f"kh{s}",
                                   name=f"kh{s}")
                    ksrc = kvt_s[0:64, :].rearrange("p (c e a) -> p c e a",
                                                    c=2, e=2)[:, :, 1, :]
                    nc.scalar.dma_start(out=kh[64:128, :, :], in_=ksrc)
                    k2hi[s] = kh
                    yield

            def q_units(p):
                # Q-lo (PE cols 0-63) || Q-hi dup (PE cols 64-127)
                xp = xtiles[p]
                ps_qk = pproj.tile([128, 512], f32, tag="q", name="psqk")
                for d in range(DC):
                    rhs_e = xp[:, d, :].rearrange("p (b e c) -> p e b c",
                                                  e=2, c=128)[:, 0, :, :]
                    st, sp = (d == 0), (d == DC - 1)
                    nc.tensor.matmul(ps_qk[0:64, :], wq_sb[:, d, :],
                                     rhs_e, start=st, stop=sp)
                    nc.tensor.matmul(ps_qk[64:128, :], wq_sb[:, d, :],
                                     rhs_e, start=st, stop=sp)
                    if d == 3:
                        yield
                yield
                qt_p = qtp.tile([128, 512], f16, tag=f"qt{p}", name=f"qt{p}")
                nc.vector.tensor_scalar_add(qt_p[0:64, :], ps_qk[0:64, :],
                                            bqd_sb[0:64, :])
                nc.vector.tensor_scalar_add(qt_p[64:128, :], ps_qk[64:128, :],
                                            bqd_sb[64:128, :])
                qt[p] = qt_p
                yield

            def proj_units(p):
                """Projection work units (~1-2us tensor each) for pair p,
                interleaved into the previous attention group's scalar-bound
                visit cycles."""
                yield from kv_units(p, 0)
                yield from kv_units(p, 1)
                yield from q_units(p)

            def run_units(g):
                if g is None:
                    return
                for _ in g:
                    pass

            def attn_group(qi, bg=None):
                o_acc = pacc.tile([128, 512], f32, tag="oacc", name=f"oacc{qi}")
                den = denp.tile([128, 2, 512], f16, tag="den", name=f"den{qi}")
                nc.vector.memset(den[:], 0.0)
                nvis = 4 * qi + 4
                pend = []

                def emit_av(v):
                    u, su, n, pt = v
                    first, last = (u == 0), (u == nvis - 1)
                    for i in range(2):
                        c = 2 * u + i
                        s, cc = c // 4, c % 4
                        nc.tensor.matmul(
                            o_acc[i * 64:(i + 1) * 64, su * 128:512],
                            vn[s][:, cc, :], pt[:, i, 0:n],
                            start=first, stop=last)

                for u in range(nvis):
                    su = max(0, u - 4 * qi)
                    n = (4 - su) * 128
                    diag = u >= 4 * qi
                    ps_s = pmm.tile([128, 2, 512], f32, tag="mm", name="pss")
                    ce, co = 2 * u, 2 * u + 1
                    se, a = ce // 4, (ce % 4) * 128
                    nc.tensor.matmul(ps_s[:, 0, 0:n],
                                     kvt[se][0:64, a:a + 128],
                                     qt[qi][0:64, su * 128:512],
                                     start=True, stop=True)
                    nc.tensor.matmul(ps_s[:, 1, 0:n],
                                     k2hi[co // 4][64:128, (co % 4) // 2, :],
                                     qt[qi][64:128, su * 128:512],
                                     start=True, stop=True)
                    pt = ptp.tile([128, 2, 512], f16, tag="pt", name="pt")
                    nc.scalar.activation(pt[:, :, 0:n], ps_s[:, :, 0:n],
                                         func=mybir.ActivationFunctionType.Exp,
                                         scale=SCALE)
                    if diag:
                        nc.vector.tensor_mul(pt[:, 0, 0:128], pt[:, 0, 0:128],
                                             msk_sb[:, 0, :])
                        nc.vector.tensor_mul(pt[:, 1, 0:128], pt[:, 1, 0:128],
                                             msk_sb[:, 1, :])
                    nc.vector.tensor_add(den[:, :, su * 128:512],
                                         den[:, :, su * 128:512],
                                         pt[:, :, 0:n])
                    pend.append((u, su, n, pt))
                    if bg is not None:
                        next(bg, None)
                        if nvis <= 4:
                            next(bg, None)
                    if len(pend) > 1:
                        emit_av(pend.pop(0))
                while pend:
                    emit_av(pend.pop(0))
                if bg is not None:
                    for _ in bg:
                        pass

                ps_den = pmm.tile([1, 512], f32, tag="mm", name="psden")
                for i in range(2):
                    nc.tensor.matmul(ps_den[:], ones_sb[:], den[:, i, :],
                                     start=(i == 0), stop=(i == 1))
                o_sb = osbp.tile([128, 512], f32, tag="osb", name="osb")
                nc.vector.tensor_copy(out=o_sb[:], in_=o_acc[:])
                den_sb = osbp.tile([1, 512], f32, tag="densb", name="densb")
                nc.vector.tensor_copy(out=den_sb[:], in_=ps_den[:])
                nc.sync.dma_start(out=out[qi, 0:128, :], in_=o_sb[:])
                nc.sync.dma_start(out=out[qi, 128:129, :], in_=den_sb[:])

            from itertools import chain
            xtiles[0] = dma_x(0, nsplit=2, eng=nc.scalar)
            xtiles[1] = dma_x(1)
            run_units(proj_units(0))
            xtiles[2] = dma_x(2)
            xtiles[3] = dma_x(3)
            attn_group(0, proj_units(1))
            attn_group(1, proj_units(2))
            attn_group(2, proj_units(3))
            attn_group(3)

    nc.compile()
    return nc


def get_nc():
    if "nc" not in _cached:
        _cached["nc"] = _build_nc()
    return _cached["nc"]


def _masks_for_half(j):
    """(128, 2, 128) first-block mask patterns, indexed by chunk parity.

    slot 0 (even local chunk): tril for both cores.
    slot 1 (odd local chunk): zeros for core 0 (block strictly above the
    diagonal there), ones for core 1 (strictly below).
    """
    p = np.arange(128)[:, None]
    jj = np.arange(128)[None, :]
    tril = (jj >= p).astype(np.float32)        # K^T layout: k on partitions
    m = np.empty((128, 2, 128), dtype=np.float32)
    m[:, 0, :] = tril
    m[:, 1, :] = 1.0 if j == 1 else 0.0
    return m.astype(np.float16)


def prepare_in_maps(x, Wk, bk, Wq, bq, Wv, bv):
    wkv = np.ascontiguousarray(
        np.concatenate([Wk, Wv], axis=1).reshape(DC, 128, 128)
        .transpose(1, 0, 2)).astype(BF16)
    wq_c = np.ascontiguousarray(
        Wq.reshape(DC, 128, H).transpose(1, 0, 2)).astype(BF16)
    bkv = np.concatenate([bk, bv]).reshape(128, 1).astype(np.float32)
    bkk = np.concatenate([bk, bk]).reshape(128, 1).astype(np.float32)
    bqd = np.concatenate([bq, bq]).reshape(128, 1).astype(np.float32)
    masks = [_masks_for_half(0), _masks_for_half(1)]

    swap = np.arange(NKC).reshape(-1, 2)[:, ::-1].reshape(-1)  # pair-swap blocks
    in_maps = []
    for core in range(N_CORES):
        b, j = core // 2, core % 2
        xTb = x[b].T                                          # (D, S)
        if j == 1:
            xTb = xTb.reshape(D, NKC, 128)[:, swap, :].reshape(D, S)
        # tile layout: (s-pair, 128, d-chunk, 1024) contiguous
        xTb = np.ascontiguousarray(
            xTb.reshape(DC, 128, SP, 1024).transpose(2, 1, 0, 3)
        ).astype(BF16)
        in_maps.append({
            "xT": xTb, "wkv": wkv, "wq": wq_c, "bkv": bkv, "bkk": bkk,
            "bqd": bqd, "msk": masks[j],
        })
    return in_maps


def assemble_output(results):
    """results: list of 8 dicts with 'out' (NQT, 129, 512) -> full (B, S, H)."""
    out = np.empty((B, S, H), dtype=np.float32)
    for core in range(N_CORES):
        b, j = core // 2, core % 2
        loc = results[core]["out"]                       # (NQT, 129, 512)
        o = (loc[:, 0:64, :] + loc[:, 64:128, :]) / loc[:, 128:129, :]
        ob = o.reshape(NQT, H, 4, 128).transpose(0, 2, 3, 1)  # (qi, bi, 128, H)
        full = out[b].reshape(NKC, 128, H)
        for qi in range(NQT):
            for bi in range(4):
                full[8 * qi + 2 * bi + j] = ob[qi, bi]
    return out


def run_sharded(inputs, trace=False, trace_kwargs=None):
    from concourse.bass_utils import run_bass_kernel_spmd

    x = np.asarray(inputs["x"], dtype=np.float32)
    in_maps = prepare_in_maps(
        x,
        np.asarray(inputs["Wk"], dtype=np.float32),
        np.asarray(inputs["bk"], dtype=np.float32),
        np.asarray(inputs["Wq"], dtype=np.float32),
        np.asarray(inputs["bq"], dtype=np.float32),
        np.asarray(inputs["Wv"], dtype=np.float32),
        np.asarray(inputs["bv"], dtype=np.float32),
    )
    nc = get_nc()
    kw = {}
    if trace:
        kw["trace"] = True
        if trace_kwargs:
            kw.update(trace_kwargs)
    res = run_bass_kernel_spmd(nc, in_maps, core_ids=list(range(N_CORES)), **kw)
    return assemble_output(res.results), res


def kernel(**inputs):
    out, _ = run_sharded(inputs)
    return out


# revision 35
# speedup vs baseline: 1.0721x; 1.0334x over previous
"""Single-head causal attention (B=4, S=4096, D=1024, H=64) on 8 TRN2 NeuronCores.

Sharding: 2 cores per batch. Query rows are split between the pair by
interleaving 128-row blocks (core j takes blocks with parity j). The host
pair-swaps the columns of x^T for odd cores so every core runs the IDENTICAL
instruction stream (SPMD); causal asymmetry is absorbed into a tiny
(128, 2, 128) per-core mask constant.

DMA discipline: the hardware DGE rings are in-order, so a ring carrying a
transfer that WAITS on upstream compute head-of-line-blocks everything behind
it. The Sync ring therefore carries only wait-free bulk (weights, x pairs
1-3) plus tail outputs; x pair 0 rides the Scalar ring. Every tensor that
needs partition-shifted copies is instead produced directly at the right
partitions by col-tiled projection matmuls.

Device algorithm per core (f32 PSUM accumulate):
  per s-pair p (emitted as ~1-2us work units interleaved into the PREVIOUS
  attention group's scalar-bound visit cycles, so TensorE projects pair p+1
  while ScalarE exponentiates group p):
    [K^T;V^T] = [Wk|Wv]^T @ x^T  (16 full-width matmuls)
    Q^T duplicated onto both partition halves by col-tiled matmul pairs
    (PE cols 0-63 / 64-127 stream concurrently)
    odd-chunk K^T copied to partitions 64-127 via the Scalar DMA ring
    vn (V natural) via PE-transpose into a shared PSUM ring slot -> SBUF.
  attention for q-tile qi (after proj pair qi), chunk-pair u, lag-1 pipeline:
    scores: TWO row-tiled concurrent matmuls (PE rows 0-63: even chunk from
            kvt, rows 64-127: odd chunk from k2hi) -> (128, 2, N) PSUM slab
    P = exp(S/8) -> f16 (ScalarE; scores bounded, no max subtraction)
    diagonal slabs: first-block mask multiply (DVE)
    den (128, 2, 512) f16 += P (DVE), reduced by one ones-matmul per group
    AV: TWO col-tiled concurrent matmuls (PE cols 0-63: even chunk -> o_acc
        rows 0-63, cols 64-127: odd -> rows 64-127)
  out[qi] = (129, 512) f32: rows 0-127 raw o_acc halves, row 128 den.
  Host: O = (rows 0:64 + rows 64:128) / row 128, transpose to (q, h).
"""

import sys

for _p in ("/opt/trn_rl_repo", "/root/.axon_site"):
    if _p not in sys.path:
        sys.path.insert(0, _p)

import numpy as np
import ml_dtypes

B, S, D, H = 4, 4096, 1024, 64
N_CORES = 8
DC = D // 128          # 8 d-chunks
ST = S // 512          # 8 s-tiles of 512
SP = ST // 2           # 4 s-pairs of 1024
NKC = S // 128         # 32 k-chunks of 128
NQT = 4                # q-tiles of 512 per core
SCALE = 1.0 / 8.0      # 1/sqrt(H)

BF16 = ml_dtypes.bfloat16

_cached = {}


def _build_nc():
    from concourse import bacc, tile, mybir
    from concourse.masks import make_identity

    f32 = mybir.dt.float32
    bf16 = mybir.dt.bfloat16
    f16 = mybir.dt.float16

    nc = bacc.Bacc("TRN2", target_bir_lowering=False, debug=False,
                   num_devices=N_CORES)

    xT = nc.declare_dram_parameter("xT", [SP, 128, DC, 1024], bf16, isOutput=False)
    wkv = nc.declare_dram_parameter("wkv", [128, DC, 128], bf16, isOutput=False)
    wq = nc.declare_dram_parameter("wq", [128, DC, H], bf16, isOutput=False)
    bkv = nc.declare_dram_parameter("bkv", [128, 1], f32, isOutput=False)
    bkk = nc.declare_dram_parameter("bkk", [128, 1], f32, isOutput=False)
    bqd = nc.declare_dram_parameter("bqd", [128, 1], f32, isOutput=False)
    msk = nc.declare_dram_parameter("msk", [128, 2, 128], f16, isOutput=False)
    out = nc.declare_dram_parameter("out", [NQT, 129, 512], f32, isOutput=True)

    with tile.TileContext(nc) as tc:
        with (
            tc.tile_pool(name="consts", bufs=1) as consts,
            tc.tile_pool(name="xtp", bufs=1) as xtp,
            tc.tile_pool(name="kvtp", bufs=1) as kvtp,
            tc.tile_pool(name="khip", bufs=1) as khip,
            tc.tile_pool(name="vnp", bufs=1) as vnp,
            tc.tile_pool(name="qtp", bufs=1) as qtp,
            tc.tile_pool(name="ptp", bufs=6) as ptp,
            tc.tile_pool(name="denp", bufs=2) as denp,
            tc.tile_pool(name="osbp", bufs=2) as osbp,
            tc.tile_pool(name="pproj", bufs=1, space="PSUM") as pproj,
            tc.tile_pool(name="pmm", bufs=2, space="PSUM") as pmm,
            tc.tile_pool(name="pacc", bufs=2, space="PSUM") as pacc,
        ):
            wkv_sb = consts.tile([128, DC, 128], bf16)
            for d in range(DC):   # 8 queues in parallel: ~1.5us, not ~10us
                nc.sync.dma_start(out=wkv_sb[:, d, :], in_=wkv[:, d, :])
            bkv_sb = consts.tile([128, 1], f32)
            nc.sync.dma_start(out=bkv_sb[:], in_=bkv[:, :])
            bkk_sb = consts.tile([128, 1], f32)
            nc.sync.dma_start(out=bkk_sb[:], in_=bkk[:, :])
            bqd_sb = consts.tile([128, 1], f32)
            nc.sync.dma_start(out=bqd_sb[:], in_=bqd[:, :])
            wq_sb = consts.tile([128, DC, H], bf16)
            for d in range(0, DC, 2):
                nc.sync.dma_start(out=wq_sb[:, d:d + 2, :], in_=wq[:, d:d + 2, :])
            msk_sb = consts.tile([128, 2, 128], f16)
            nc.sync.dma_start(out=msk_sb[:], in_=msk[:, :, :])
            ones_sb = consts.tile([128, 1], f16)
            nc.vector.memset(ones_sb[:], 1.0)
            ident = consts.tile([128, 128], f16)
            make_identity(nc, ident[:, :])

            kvt = {}   # s-tile -> (128, 512) f16: rows 0-63 K^T, 64-127 V^T
            k2hi = {}  # s-tile -> (128, 2, 128) f16 rows 64+: odd-chunk K^T
            vn = {}    # s-tile -> (128, 4, 64) f16 V natural
            qt = {}    # qi -> (128, 512) f16 Q^T on both partition halves

            def dma_x(p, nsplit=1, eng=None):
                eng = eng or nc.sync
                xp = xtp.tile([128, DC, 1024], bf16, tag=f"x{p}", name=f"x{p}")
                w = 1024 // nsplit
                for d in range(DC):
                    for h in range(nsplit):
                        eng.dma_start(out=xp[:, d, h * w:(h + 1) * w],
                                      in_=xT[p, :, d, h * w:(h + 1) * w])
                return xp

            xtiles = {}

            def kv_units(p, sh):
                xp = xtiles[p]
                if True:
                    s = 2 * p + sh
                    ps_kv = pproj.tile([128, 512], f32, tag="kv", name="pskv")
                    for d in range(DC):
                        nc.tensor.matmul(ps_kv[:], wkv_sb[:, d, :],
                                         xp[:, d, sh * 512:(sh + 1) * 512],
                                         start=(d == 0), stop=(d == DC - 1))
                    yield
                    kvt_s = kvtp.tile([128, 512], f16, tag=f"kvt{s}",
                                      name=f"kvt{s}")
                    nc.vector.tensor_scalar_add(kvt_s[:], ps_kv[:], bkv_sb[:, :])
                    kvt[s] = kvt_s
                    # V natural via PE transpose (PSUM ring slot -> SBUF)
                    ps_t = pmm.tile([128, 4, 64], f16, tag="mm", name="pst")
                    for cc in range(4):
                        nc.tensor.transpose(ps_t[:, cc, :],
                                            kvt_s[64:128, cc * 128:(cc + 1) * 128],
                                            ident[64:128, 64:128])
                    vn_s = vnp.tile([128, 4, 64], f16, tag=f"vn{s}",
                                    name=f"vn{s}")
                    nc.vector.tensor_copy(out=vn_s[:], in_=ps_t[:])
                    vn[s] = vn_s
                    # odd-chunk K^T at partitions 64-127 (scalar ring is
                    # empty after x0, so this never head-of-line blocks)
                    kh = khip.tile([128, 2, 128], f16, tag=f"kh{s}",
                                   name=f"kh{s}")
                    ksrc = kvt_s[0:64, :].rearrange("p (c e a) -> p c e a",
                                                    c=2, e=2)[:, :, 1, :]
                    nc.scalar.dma_start(out=kh[64:128, :, :], in_=ksrc)
                    k2hi[s] = kh
                    yield

            def q_units(p):
                # Q-lo (PE cols 0-63) || Q-hi dup (PE cols 64-127)
                xp = xtiles[p]
                ps_qk = pproj.tile([128, 512], f32, tag="q", name="psqk")
                for d in range(DC):
                    rhs_e = xp[:, d, :].rearrange("p (b e c) -> p e b c",
                                                  e=2, c=128)[:, 0, :, :]
                    st, sp = (d == 0), (d == DC - 1)
                    nc.tensor.matmul(ps_qk[0:64, :], wq_sb[:, d, :],
                                     rhs_e, start=st, stop=sp)
                    nc.tensor.matmul(ps_qk[64:128, :], wq_sb[:, d, :],
                                     rhs_e, start=st, stop=sp)
                    if d == 3:
                        yield
                yield
                qt_p = qtp.tile([128, 512], f16, tag=f"qt{p}", name=f"qt{p}")
                nc.vector.tensor_scalar_add(qt_p[0:64, :], ps_qk[0:64, :],
                                            bqd_sb[0:64, :])
                nc.vector.tensor_scalar_add(qt_p[64:128, :], ps_qk[64:128, :],
                                            bqd_sb[64:128, :])
                qt[p] = qt_p
                yield

            def proj_units(p):
                """Projection work units (~1-2us tensor each) for pair p,
                interleaved into the previous attention group's scalar-bound
                visit cycles."""
                yield from kv_units(p, 0)
                yield from kv_units(p, 1)
                yield from q_units(p)

            def run_units(g):
                if g is None:
                    return
                for _ in g:
                    pass

            def attn_group(qi, bg=None):
                o_acc = pacc.tile([128, 512], f32, tag="oacc", name=f"oacc{qi}")
                den = denp.tile([128, 2, 512], f16, tag="den", name=f"den{qi}")
                nc.vector.memset(den[:], 0.0)
                nvis = 4 * qi + 4
                pend = []

                def emit_av(v):
                    u, su, n, pt = v
                    first, last = (u == 0), (u == nvis - 1)
                    for i in range(2):
                        c = 2 * u + i
                        s, cc = c // 4, c % 4
                        nc.tensor.matmul(
                            o_acc[i * 64:(i + 1) * 64, su * 128:512],
                            vn[s][:, cc, :], pt[:, i, 0:n],
                            start=first, stop=last)

                for u in range(nvis):
                    su = max(0, u - 4 * qi)
                    n = (4 - su) * 128
                    diag = u >= 4 * qi
                    ps_s = pmm.tile([128, 2, 512], f32, tag="mm", name="pss")
                    ce, co = 2 * u, 2 * u + 1
                    se, a = ce // 4, (ce % 4) * 128
                    nc.tensor.matmul(ps_s[:, 0, 0:n],
                                     kvt[se][0:64, a:a + 128],
                                     qt[qi][0:64, su * 128:512],
                                     start=True, stop=True)
                    nc.tensor.matmul(ps_s[:, 1, 0:n],
                                     k2hi[co // 4][64:128, (co % 4) // 2, :],
                                     qt[qi][64:128, su * 128:512],
                                     start=True, stop=True)
                    pt = ptp.tile([128, 2, 512], f16, tag="pt", name="pt")
                    nc.scalar.activation(pt[:, :, 0:n], ps_s[:, :, 0:n],
                                         func=mybir.ActivationFunctionType.Exp,
                                         scale=SCALE)
                    if diag:
                        nc.vector.tensor_mul(pt[:, 0, 0:128], pt[:, 0, 0:128],
                                             msk_sb[:, 0, :])
                        nc.vector.tensor_mul(pt[:, 1, 0:128], pt[:, 1, 0:128],
                                             msk_sb[:, 1, :])
                    nc.vector.tensor_add(den[:, :, su * 128:512],
                                         den[:, :, su * 128:512],
                                         pt[:, :, 0:n])
                    pend.append((u, su, n, pt))
                    if bg is not None:
                        next(bg, None)
                        if nvis <= 4:
                            next(bg, None)
                    if len(pend) > 1:
                        emit_av(pend.pop(0))
                while pend:
                    emit_av(pend.pop(0))
                if bg is not None:
                    for _ in bg:
                        pass

                ps_den = pmm.tile([1, 512], f32, tag="mm", name="psden")
                for i in range(2):
                    nc.tensor.matmul(ps_den[:], ones_sb[:], den[:, i, :],
                                     start=(i == 0), stop=(i == 1))
                o_sb = osbp.tile([128, 512], f32, tag="osb", name="osb")
                nc.vector.tensor_copy(out=o_sb[:], in_=o_acc[:])
                den_sb = osbp.tile([1, 512], f32, tag="densb", name="densb")
                nc.vector.tensor_copy(out=den_sb[:], in_=ps_den[:])
                nc.sync.dma_start(out=out[qi, 0:128, :], in_=o_sb[:])
                nc.sync.dma_start(out=out[qi, 128:129, :], in_=den_sb[:])

            from itertools import chain
            xtiles[0] = dma_x(0, nsplit=2, eng=nc.scalar)
            xtiles[1] = dma_x(1)
            run_units(proj_units(0))
            xtiles[2] = dma_x(2)
            xtiles[3] = dma_x(3)
            attn_group(0, proj_units(1))
            attn_group(1, proj_units(2))
            attn_group(2, proj_units(3))
            attn_group(3)

    nc.compile()
    return nc


def get_nc():
    if "nc" not in _cached:
        _cached["nc"] = _build_nc()
    return _cached["nc"]


def _masks_for_half(j):
    """(128, 2, 128) first-block mask patterns, indexed by chunk parity.

    slot 0 (even local chunk): tril for both cores.
    slot 1 (odd local chunk): zeros for core 0 (block strictly above the
    diagonal there), ones for core 1 (strictly below).
    """
    p = np.arange(128)[:, None]
    jj = np.arange(128)[None, :]
    tril = (jj >= p).astype(np.float32)        # K^T layout: k on partitions
    m = np.empty((128, 2, 128), dtype=np.float32)
    m[:, 0, :] = tril
    m[:, 1, :] = 1.0 if j == 1 else 0.0
    return m.astype(np.float16)


def prepare_in_maps(x, Wk, bk, Wq, bq, Wv, bv):
    wkv = np.ascontiguousarray(
        np.concatenate([Wk, Wv], axis=1).reshape(DC, 128, 128)
        .transpose(1, 0, 2)).astype(BF16)
    wq_c = np.ascontiguousarray(
        Wq.reshape(DC, 128, H).transpose(1, 0, 2)).astype(BF16)
    bkv = np.concatenate([bk, bv]).reshape(128, 1).astype(np.float32)
    bkk = np.concatenate([bk, bk]).reshape(128, 1).astype(np.float32)
    bqd = np.concatenate([bq, bq]).reshape(128, 1).astype(np.float32)
    masks = [_masks_for_half(0), _masks_for_half(1)]

    swap = np.arange(NKC).reshape(-1, 2)[:, ::-1].reshape(-1)  # pair-swap blocks
    in_maps = []
    for core in range(N_CORES):
        b, j = core // 2, core % 2
        xTb = x[b].T                                          # (D, S)
        if j == 1:
            xTb = xTb.reshape(D, NKC, 128)[:, swap, :].reshape(D, S)
        # tile layout: (s-pair, 128, d-chunk, 1024) contiguous
        xTb = np.ascontiguousarray(
            xTb.reshape(DC, 128, SP, 1024).transpose(2, 1, 0, 3)
        ).astype(BF16)
        in_maps.append({
            "xT": xTb, "wkv": wkv, "wq": wq_c, "bkv": bkv, "bkk": bkk,
            "bqd": bqd, "msk": masks[j],
        })
    return in_maps


def assemble_output(results):
    """results: list of 8 dicts with 'out' (NQT, 129, 512) -> full (B, S, H)."""
    out = np.empty((B, S, H), dtype=np.float32)
    for core in range(N_CORES):
        b, j = core // 2, core % 2
        loc = results[core]["out"]                       # (NQT, 129, 512)
        o = (loc[:, 0:64, :] + loc[:, 64:128, :]) / loc[:, 128:129, :]
        ob = o.reshape(NQT, H, 4, 128).transpose(0, 2, 3, 1)  # (qi, bi, 128, H)
        full = out[b].reshape(NKC, 128, H)
        for qi in range(NQT):
            for bi in range(4):
                full[8 * qi + 2 * bi + j] = ob[qi, bi]
    return out


def run_sharded(inputs, trace=False, trace_kwargs=None):
    from concourse.bass_utils import run_bass_kernel_spmd

    x = np.asarray(inputs["x"], dtype=np.float32)
    in_maps = prepare_in_maps(
        x,
        np.asarray(inputs["Wk"], dtype=np.float32),
        np.asarray(inputs["bk"], dtype=np.float32),
        np.asarray(inputs["Wq"], dtype=np.float32),
        np.asarray(inputs["bq"], dtype=np.float32),
        np.asarray(inputs["Wv"], dtype=np.float32),
        np.asarray(inputs["bv"], dtype=np.float32),
    )
    nc = get_nc()
    kw = {}
    if trace:
        kw["trace"] = True
        if trace_kwargs:
            kw.update(trace_kwargs)
    res = run_bass_kernel_spmd(nc, in_maps, core_ids=list(range(N_CORES)), **kw)
    return assemble_output(res.results), res


def kernel(**inputs):
    out, _ = run_sharded(inputs)
    return out
